# revision 39
# baseline (speedup 1.0000x reference)
"""AttentionAggregator Trainium2 kernel (final).

B=20000 nodes, K=10 neighbors, N=100000 embed rows, F=256, H=128.
Data-parallel over B across 8 NeuronCores (2500 nodes/core).

Per-core device pipeline (all matmuls bf16 -> fp32 PSUM):
  - transposed gather: dma_gather(transpose=True) from a per-core
    compacted bf16 embedding table (int16 indices) -> X^T in SBUF
    as [128 f-half, 2, rows]; four gathers per 1920-row chunk
    (512/512/512/384 - SWDGE faults above 512 rows per gather)
  - startup: chunks 0/1 arrive pre-gathered+transposed from the host
    over both HWDGE rings (ACT + SP), so the PE starts at ~10us instead
    of waiting for the ~17.5us GPSIMD library load + first gather
  - T_j^T = tanh(W_ja^T X^T)      (j=1,2,3; two f-half matmuls + one
    ACT tanh per gather piece, pieces-outer so G is ready early)
  - G = (W2b W1b^T) T1^T          (const stationary CmT = W1b W2b^T),
    PSUM->SBUF bf16 copy on DVE
  - per 12-node group g (120 rows): S block = G_g.T @ T2_g with a
    rank-13 constant matmul adding -50 off-block (mask); S layout is
    [query-part x key-col]
  - E = exp(S)  (ACT, PSUM->SBUF bf16)
  - r = row-sums of E per group (DVE reduce), rec = 1/r (bf16)
  - c^T = E_g (stationary) @ rec_g  -> column weights [120 x 1] in PSUM
  - M = V * c in one broadcast tensor_tensor (V_g = T3_g^T W3b via PE
    transpose+proj; c staged through SBUF - walrus rejects PSUM in1)
  - out_g = bi^T @ M: one 512-col matmul per quad (bi maps row j ->
    node j//10 identically in all 4 groups); out stage deferred to
    chunk end so its copies don't head-of-line block the DVE queue;
    bf16 output staging, host casts to f32
  - PSUM banks 3/3/1/1 (FL/S/V/out): deep FL + S pipelining is worth
    more than double-buffered V/out
  - tail chunk (48 nodes) is software-pipelined into chunk 12
"""

import sys

sys.path.insert(0, "/opt/trn_rl_repo")

import numpy as np
import ml_dtypes

import concourse.bass as bass
import concourse.bacc as bacc
import concourse.mybir as mybir
import concourse.tile as tile
from concourse.bass_utils import run_bass_kernel_spmd

BF16 = ml_dtypes.bfloat16

B, K, N, F, H = 20000, 10, 100000, 256, 128
NCORES = 8
B_CORE = B // NCORES                # 2500
CHUNK_NODES = 192                   # nodes per chunk
CHUNK_ROWS = CHUNK_NODES * K        # 1920
NCHUNK = 13                         # full chunks; + 1 tail chunk of 48 nodes
TAIL_NODES = 48                     # 1 quad (4 groups of 12)
TAIL_ROWS = 512                     # one gather (48*K=480 used, 512 padded)
PAD_NODES = NCHUNK * CHUNK_NODES + TAIL_NODES   # 2544
PAD_ROWS = NCHUNK * CHUNK_ROWS + TAIL_ROWS      # 25472
IDX_COLS = NCHUNK * (CHUNK_ROWS // 16) + TAIL_ROWS // 16  # 1592
IDX0_COLS = CHUNK_ROWS // 16        # 120 (chunk 0, separate tensor)
GSIZES = (512, 512, 512, 384)       # per-chunk gather sizes (sum 1920)
GOFFS = (0, 512, 1024, 1536)
GROUP_NODES = 12                    # nodes per attention group
GROUP_ROWS = GROUP_NODES * K        # 120
QUADS = 4                           # quads per chunk (4 groups each)
TBL_ROWS = 25600                    # per-core compact table rows (padded)
MASK_L = 50.0

_CACHED = {}


def _build_program():
    nc = bacc.Bacc(
        "TRN2",
        target_bir_lowering=False,
        debug=False,
        num_devices=NCORES,
        num_swdge_queues=4,
    )
    dt = mybir.dt
    f32, bf16, i16 = dt.float32, dt.bfloat16, dt.int16
    Tanh = mybir.ActivationFunctionType.Tanh
    Exp = mybir.ActivationFunctionType.Exp

    table = nc.dram_tensor("table", [TBL_ROWS, F], bf16, kind="ExternalInput")
    # chunks 0/1 arrive pre-gathered+transposed from the host so the PE can
    # start before the SWDGE library load (~17.5us) and first gather finish
    x0t = nc.dram_tensor("x0t", [128, 2, CHUNK_ROWS], bf16,
                         kind="ExternalInput")
    x1t = nc.dram_tensor("x1t", [128, 2, CHUNK_ROWS], bf16,
                         kind="ExternalInput")
    idxs1 = nc.dram_tensor("idxs1", [128, IDX_COLS - 2 * IDX0_COLS], i16,
                           kind="ExternalInput")
    # packed constants: 8 weight mats (needed first) | um | vm | blockind
    wnames = ["w1a0", "w1a1", "w2a0", "w2a1", "w3a0", "w3a1", "w3b", "cmt"]
    WREST = 128 + 492 + 32
    wpk_w = nc.dram_tensor("wpk_w", [128, 8 * 128], bf16,
                           kind="ExternalInput")
    wpk_r = nc.dram_tensor("wpk_r", [128, WREST], bf16, kind="ExternalInput")
    # bf16 output staging: halves the DVE copy cost and store bytes; the
    # host casts back to f32 (adds ~0.17% rel err, within tolerance)
    out = nc.dram_tensor("out", [PAD_NODES, H], bf16, kind="ExternalOutput")

    with tile.TileContext(nc) as tc:
        with (
            tc.tile_pool(name="consts", bufs=1) as cpool,
            tc.tile_pool(name="xt", bufs=4) as xpool,
            tc.tile_pool(name="tp", bufs=4) as tpool,
            tc.tile_pool(name="attn", bufs=4) as apool,
            tc.tile_pool(name="psbig", bufs=3, space="PSUM") as psbig,
            tc.tile_pool(name="pss", bufs=3, space="PSUM") as pssp,
            tc.tile_pool(name="psv", bufs=1, space="PSUM") as psvp,
            tc.tile_pool(name="pso", bufs=1, space="PSUM") as psop,
        ):
            # ---- load constants to SBUF ----
            # startup-critical tensors ride both HWDGE rings, piece by piece,
            # ordered by first use: chunk-0 x pieces + weights on ACT ring,
            # mask consts + chunk-1 x pieces + gather indices on SP ring
            x0sb = cpool.tile([128, 2, CHUNK_ROWS], bf16, tag="c_x0")
            x1sb = cpool.tile([128, 2, CHUNK_ROWS], bf16, tag="c_x1")
            wp = cpool.tile([128, 8 * 128 + WREST], bf16, tag="c_wpack")
            nc.scalar.dma_start(out=x0sb[:, :, 0:512], in_=x0t[:, :, 0:512])
            nc.scalar.dma_start(out=wp[:, 0:1024], in_=wpk_w[:, :])
            nc.sync.dma_start(out=wp[:, 1024:], in_=wpk_r[:, :])
            for o, w in zip(GOFFS[1:], GSIZES[1:]):
                nc.scalar.dma_start(out=x0sb[:, :, o:o + w],
                                    in_=x0t[:, :, o:o + w])
            for o, w in zip(GOFFS, GSIZES):
                nc.sync.dma_start(out=x1sb[:, :, o:o + w],
                                  in_=x1t[:, :, o:o + w])
            idx1_sb = cpool.tile([128, IDX_COLS - 2 * IDX0_COLS], i16,
                                 tag="c_idx1")
            nc.sync.dma_start(out=idx1_sb[:, :], in_=idxs1[:, :])
            wsb = {n: wp[:, 128 * i:128 * (i + 1)]
                   for i, n in enumerate(wnames)}
            um_sb = wp[0:13, 1024:1152]
            vm_sb = wp[0:13, 1152:1644]
            bi_sb = wp[:, 1644:1676]

            # hoisted num_idxs registers (one MOVE each instead of per-gather)
            nregs = {sz: nc.gpsimd.to_reg(sz) for sz in (512, 384)}

            def gather(xt, idx_sb, icol0, gsz, queue):
                """Gather gsz rows into the full tile xt [128, 2, gsz]."""
                nc.gpsimd.dma_gather(
                    out_ap=xt[:, :, :],
                    in_ap=table[:, :],
                    idxs_ap=idx_sb[:, icol0:icol0 + gsz // 16],
                    num_idxs=gsz,
                    num_idxs_reg=nregs[gsz],
                    elem_size=F,
                    transpose=True,
                    queue_num=queue,
                )

            def first_layer(xts, ts, widths=GSIZES, offs=GOFFS):
                """T_j = tanh(W_ja^T X^T) and G = CmT T1.

                Pieces-outer order: each gather piece is fully consumed
                (t1/t2/t3/G) before the next, so compute starts as soon as
                the first piece lands and G is ready early for the quads.
                """
                t1, t2, t3, g = ts
                for xt, w, to in zip(xts, widths, offs):
                    for w0, w1, tj in (("w1a0", "w1a1", t1),
                                       ("w2a0", "w2a1", t2),
                                       ("w3a0", "w3a1", t3)):
                        ps = psbig.tile([128, 512], f32, tag="psb")
                        nc.tensor.matmul(ps[:, 0:w], wsb[w0][:, :],
                                         xt[:, 0, 0:w],
                                         start=True, stop=False)
                        nc.tensor.matmul(ps[:, 0:w], wsb[w1][:, :],
                                         xt[:, 1, 0:w],
                                         start=False, stop=True)
                        nc.scalar.activation(tj[:, to:to + w], ps[:, 0:w],
                                             Tanh)
                    psg = psbig.tile([128, 512], f32, tag="psb")
                    nc.tensor.matmul(psg[:, 0:w], wsb["cmt"][:, :],
                                     t1[:, to:to + w],
                                     start=True, stop=True)
                    nc.vector.tensor_copy(g[:, to:to + w], psg[:, 0:w])

            def quad(t2t, t3t, gt, outst, q, row_base):
                """One quad: 4 groups of 12 nodes starting at row_base."""
                ps_s = pssp.tile([128, 492], f32, tag="pss")
                # mask bias first (start=True over all 480 cols)
                nc.tensor.matmul(ps_s[:, 0:492], um_sb[:, :], vm_sb[:, :],
                                 start=True, stop=False,
                                 skip_group_check=True)
                for g4 in range(4):
                    r0 = row_base + GROUP_ROWS * g4
                    nc.tensor.matmul(
                        ps_s[:, 120 * g4:120 * g4 + 120],
                        gt[:, r0:r0 + 128],
                        t2t[:, r0:r0 + 120],
                        start=False, stop=True, skip_group_check=True)
                em = apool.tile([128, 488], bf16, tag="em")
                # cols 480:488 hold exp(0)=1 from the mask matmul's zero
                # columns; they only feed unused ct partitions 120:127
                nc.scalar.activation(em[0:120, 0:488], ps_s[0:120, 0:488],
                                     Exp)
                rq = apool.tile([128, 4], f32, tag="rq")
                nc.vector.reduce_sum(
                    rq[0:120, :],
                    em[0:120, 0:480].rearrange("p (g j) -> p g j", g=4),
                    axis=mybir.AxisListType.X)
                rec = apool.tile([128, 4], bf16, tag="rec")
                with nc.allow_low_precision(
                        reason="1/r feeds a bf16 matmul; bf16 rounding of the "
                               "per-query softmax scale is within tolerance"):
                    nc.vector.reciprocal(rec[0:120, :], rq[0:120, :])

                # V blocks (PE transpose + W3b) and column weights c
                ps_v = psvp.tile([128, 4, 128], f32, tag="psv")
                ct_ps = ps_s[:, 488:492]
                for g4 in range(4):
                    r0 = row_base + GROUP_ROWS * g4
                    nc.tensor.matmul(ps_v[:, g4, :], t3t[:, r0:r0 + 128],
                                     wsb["w3b"][:, :],
                                     start=True, stop=True)
                    nc.tensor.matmul(ct_ps[:, g4:g4 + 1],
                                     em[0:120, 120 * g4:120 * g4 + 128],
                                     rec[0:120, g4:g4 + 1],
                                     start=True, stop=True,
                                     skip_group_check=True)
                ctsb = apool.tile([128, 4], f32, tag="ctsb")
                nc.vector.tensor_copy(ctsb[0:120, :], ct_ps[0:120, :])
                # M = V * c in one DVE op: c broadcast along h via a
                # 0-stride inner dim (walrus rejects a PSUM-sourced in1,
                # so c stages through SBUF first)
                m4 = apool.tile([128, 4, 128], bf16, tag=f"m4_{q}",
                                name="m4")
                with nc.allow_low_precision(
                        reason="M feeds a bf16 matmul; same rounding class "
                               "as a tensor_scalar scale-copy"):
                    nc.vector.tensor_tensor(
                        m4[0:120, :, :], ps_v[0:120, :, :],
                        ctsb[0:120, :].unsqueeze(2).broadcast_to(
                            (120, 4, 128)),
                        mybir.AluOpType.mult)
                # whole-tile read after the scale: empirically decouples the
                # next quad's V-matmuls from the m4 consumer chain
                vg = apool.tile([128, 4], f32, tag="vg")
                nc.vector.tensor_copy(vg[0:120, :], ps_v[0:120, :, 0])
                return m4

            def out_stage(m4, outst, q):
                # one 512-col out-matmul: bi maps row j -> node j//10 the
                # same way in every group, so all 4 groups share one lhsT.
                # Deferred to chunk end so the copy doesn't head-of-line
                # block the next quad's chain ops on the DVE queue.
                ps_o = psop.tile([32, 4, 128], f32, tag="pso")
                nc.tensor.matmul(ps_o[:, :, :],
                                 bi_sb[0:120, :],
                                 m4[0:120, :, :],
                                 start=True, stop=True,
                                 skip_group_check=True)
                with nc.allow_low_precision(
                        reason="bf16 output staging; host casts to f32"):
                    nc.vector.tensor_copy(
                        outst[0:32, q, :, :], ps_o[:, :, :])

            # zero the +8 pad cols read by the last group's 128-wide lhsT;
            # done once per pool buffer (tanh never writes past CHUNK_ROWS)
            for _ in range(4):
                t3z = tpool.tile([128, CHUNK_ROWS + 8], bf16, tag="t3")
                gz = tpool.tile([128, CHUNK_ROWS + 8], bf16, tag="g")
                nc.vector.memset(t3z[:, CHUNK_ROWS:], 0.0)
                nc.vector.memset(gz[:, CHUNK_ROWS:], 0.0)

            ttiles = None
            for c in range(NCHUNK):
                # ---- chunk input: preloaded (c<2) or 4 SWDGE gathers ----
                if c < 2:
                    xsb = x0sb if c == 0 else x1sb
                    xts = [xsb[:, :, to:to + w]
                           for w, to in zip(GSIZES, GOFFS)]
                else:
                    xts = [xpool.tile([128, 2, w], bf16, tag=f"xt{ti}",
                                      name=f"xt{ti}")
                           for ti, w in enumerate(GSIZES)]
                    icol = (c - 2) * 120
                    for ti, (xt, w, to) in enumerate(zip(xts, GSIZES,
                                                         GOFFS)):
                        gather(xt, idx1_sb, icol + to // 16, w, ti)
                if c == NCHUNK - 1:
                    # tail-chunk gather rides along with the last full chunk
                    xtt = xpool.tile([128, 2, 512], bf16, tag="xtt")
                    gather(xtt, idx1_sb, (NCHUNK - 2) * 120, 512, 0)

                # ---- first layers: T_j^T = tanh(W_ja^T X^T), G ----
                t1 = tpool.tile([128, CHUNK_ROWS + 8], bf16, tag="t1")
                t2 = tpool.tile([128, CHUNK_ROWS + 8], bf16, tag="t2")
                t3 = tpool.tile([128, CHUNK_ROWS + 8], bf16, tag="t3")
                g = tpool.tile([128, CHUNK_ROWS + 8], bf16, tag="g")
                first_layer(xts, (t1, t2, t3, g))
                if c == NCHUNK - 1:
                    # tail-chunk first layer, emitted before the last quads
                    # so the PE has fill work during their serial chain
                    tt1 = tpool.tile([128, 520], bf16, tag="tt1")
                    tt2 = tpool.tile([128, 520], bf16, tag="tt2")
                    tt3 = tpool.tile([128, 520], bf16, tag="tt3")
                    tg = tpool.tile([128, 520], bf16, tag="tg")
                    first_layer([xtt], (tt1, tt2, tt3, tg),
                                widths=(512,), offs=(0,))
                    ttiles = (tt2, tt3, tg)

                # ---- attention: 4 quads of 4 groups ----
                # node layout: slab g4 holds partitions 32g4..32g4+12, one
                # 128-col block per quad; host permutes nodes so slab g4 maps
                # to contiguous out rows [48g4, 48(g4+1)) of the chunk
                outst = apool.tile([32, QUADS, 4, H], bf16, tag="outst")
                m4s = [quad(t2, t3, g, outst, q, 480 * q)
                       for q in range(QUADS)]
                for q in range(QUADS):
                    out_stage(m4s[q], outst, q)

                # ---- store chunk output: one DMA per 12-row slab ----
                for g4 in range(4):
                    dst = out[c * CHUNK_NODES + 48 * g4:
                              c * CHUNK_NODES + 48 * (g4 + 1), :].rearrange(
                        "(q i) d -> i q d", i=GROUP_NODES)
                    src = outst[0:GROUP_NODES, :, g4, :]
                    nc.sync.dma_start(out=dst, in_=src)

            # ---- tail chunk: 48 real+pad nodes (1 quad) ----
            tt2, tt3, tg = ttiles
            toutst = apool.tile([32, QUADS, 4, H], bf16, tag="outst")
            tm4 = quad(tt2, tt3, tg, toutst, 0, 0)
            out_stage(tm4, toutst, 0)
            for g4 in range(4):
                dst = out[NCHUNK * CHUNK_NODES + GROUP_NODES * g4:
                          NCHUNK * CHUNK_NODES + GROUP_NODES * (g4 + 1), :]
                nc.scalar.dma_start(
                    out=dst,
                    in_=toutst[0:GROUP_NODES, 0, g4, :])

    nc.finalize()
    return nc


def _host_prep(neighbors, embed_table, W1a, W1b, W2a, W2b, W3a, W3b):
    """Shard + build per-core input maps."""
    embed_table = np.asarray(embed_table)
    ebf = np.ascontiguousarray(embed_table.astype(BF16))

    def b(x):
        return np.ascontiguousarray(np.asarray(x).astype(BF16))

    w1a, w2a, w3a = (np.asarray(w, np.float32) for w in (W1a, W2a, W3a))
    wmats = [
        w1a[0:128], w1a[128:256], w2a[0:128], w2a[128:256],
        w3a[0:128], w3a[128:256], np.asarray(W3b, np.float32),
        np.asarray(W1b, np.float32) @ np.asarray(W2b, np.float32).T,
    ]
    # mask = U @ Vm^T adds 0 in-block, -L off-block (rank 13)
    bi = np.zeros((120, GROUP_NODES), np.float32)
    for p in range(120):
        bi[p, p // K] = 1.0
    um = np.zeros((128, 128), np.float32)
    um[0:12, 0:120] = bi.T
    um[12, 0:120] = 1.0
    vm = np.zeros((128, 492), np.float32)
    for qq in range(4):
        vm[0:12, 120 * qq:120 * (qq + 1)] = MASK_L * bi.T
        vm[12, 120 * qq:120 * (qq + 1)] = -MASK_L
    bi128 = np.zeros((128, 32), np.float32)
    bi128[0:120, 0:GROUP_NODES] = bi
    wpk_w_arr = np.concatenate(wmats, axis=1)
    wpk_r_arr = np.concatenate([um, vm, bi128], axis=1)
    shared = {"wpk_w": b(wpk_w_arr), "wpk_r": b(wpk_r_arr)}

    nbr = np.asarray(neighbors).astype(np.int64)
    in_maps = []
    for c in range(NCORES):
        nb_c = nbr[c * B_CORE:(c + 1) * B_CORE]           # [2500, 10]
        uniq, inv = np.unique(nb_c, return_inverse=True)
        assert uniq.size <= TBL_ROWS
        tbl = np.zeros((TBL_ROWS, F), BF16)
        tbl[:uniq.size] = ebf[uniq]
        # permute nodes within full chunks so the packed-psum output slabs
        # land on contiguous out rows: slot 12*(4q+qq)+j <- node 48qq+12q+j
        perm = np.empty(CHUNK_NODES, np.int64)
        for pq in range(4):
            for pqq in range(4):
                for pj in range(GROUP_NODES):
                    perm[GROUP_NODES * (4 * pq + pqq) + pj] = \
                        48 * pqq + GROUP_NODES * pq + pj
        nodes = np.zeros((PAD_NODES, K), np.int16)
        nodes[:B_CORE] = inv.astype(np.int16).reshape(B_CORE, K)
        for ch in range(NCHUNK):
            blk = nodes[ch * CHUNK_NODES:(ch + 1) * CHUNK_NODES].copy()
            nodes[ch * CHUNK_NODES:(ch + 1) * CHUNK_NODES] = blk[perm]
        flat = np.zeros(PAD_ROWS, np.int16)
        flat[:nodes.size] = nodes.ravel()
        # wrap: index j of a chunk at [j % 16, j // 16], replicated to 128
        idx128 = np.zeros((128, IDX_COLS), np.int16)
        col = row = 0
        for sz in [CHUNK_ROWS] * NCHUNK + [TAIL_ROWS]:
            blk = flat[row:row + sz].reshape(sz // 16, 16).T
            idx128[:, col:col + sz // 16] = np.tile(blk, (8, 1))
            row += sz
            col += sz // 16
        # pre-gather + transpose chunks 0/1 on the host: [128, 2, 1920]
        xts_host = []
        for ch in range(2):
            rows = nodes[ch * CHUNK_NODES:(ch + 1) * CHUNK_NODES].ravel()
            xg = tbl[rows.astype(np.int64)]            # [1920, 256] bf16
            xts_host.append(np.ascontiguousarray(
                xg.reshape(CHUNK_ROWS, 2, 128).transpose(2, 1, 0)))
        in_maps.append({
            "table": tbl,
            "x0t": xts_host[0],
            "x1t": xts_host[1],
            "idxs1": np.ascontiguousarray(idx128[:, 2 * IDX0_COLS:]),
            **{k: v for k, v in shared.items()},
        })
    return in_maps


def kernel(neighbors, embed_table, W1a, W1b, W2a, W2b, W3a, W3b, _trace=False,
           **trace_kwargs):
    key = "prog"
    if key not in _CACHED:
        _CACHED[key] = _build_program()
    nc = _CACHED[key]
    in_maps = _host_prep(neighbors, embed_table, W1a, W1b, W2a, W2b, W3a, W3b)
    res = None
    for attempt in range(3):
        try:
            res = run_bass_kernel_spmd(nc, in_maps, list(range(NCORES)),
                                       trace=_trace, **trace_kwargs)
            break
        except Exception:
            # the axon/TRN2 device occasionally wedges transiently
            # (NRT_EXEC_UNIT_UNRECOVERABLE / INTERNAL); a retry recovers
            if attempt == 2:
                raise
            import time
            time.sleep(5)
    outs = [res.results[c]["out"][:B_CORE] for c in range(NCORES)]
    full = np.concatenate(outs, axis=0).astype(np.float32)
    kernel.last_results = res
    return full


# revision 40
# speedup vs baseline: 1.1898x; 1.1898x over previous
"""AttentionAggregator Trainium2 kernel (final).

B=20000 nodes, K=10 neighbors, N=100000 embed rows, F=256, H=128.
Data-parallel over B across 8 NeuronCores (2500 nodes/core).

Per-core device pipeline (all matmuls bf16 -> fp32 PSUM):
  - transposed gather: dma_gather(transpose=True) from a per-core
    compacted bf16 embedding table (int16 indices) -> X^T in SBUF
    as [128 f-half, 2, rows]; four gathers per 1920-row chunk
    (512/512/512/384 - SWDGE faults above 512 rows per gather)
  - startup: chunks 0/1 arrive pre-gathered+transposed from the host
    over both HWDGE rings (ACT + SP), so the PE starts at ~10us instead
    of waiting for the ~17.5us GPSIMD library load + first gather
  - T_j^T = tanh(W_ja^T X^T)      (j=1,2,3; two f-half matmuls + one
    ACT tanh per gather piece, pieces-outer so G is ready early)
  - G = (W2b W1b^T) T1^T          (const stationary CmT = W1b W2b^T),
    PSUM->SBUF bf16 copy on DVE
  - per 12-node group g (120 rows): S block = G_g.T @ T2_g with a
    rank-13 constant matmul adding -50 off-block (mask); S layout is
    [query-part x key-col]
  - E = exp(S)  (ACT, PSUM->SBUF bf16)
  - r = row-sums of E per group (DVE reduce), rec = 1/r (bf16)
  - c^T = E_g (stationary) @ rec_g  -> column weights [120 x 1] in PSUM
  - M = V * c in one broadcast tensor_tensor (V_g = T3_g^T W3b via PE
    transpose+proj; c staged through SBUF - walrus rejects PSUM in1)
  - out_g = bi^T @ M: one 512-col matmul per quad (bi maps row j ->
    node j//10 identically in all 4 groups); out stage deferred to
    chunk end so its copies don't head-of-line block the DVE queue;
    bf16 output staging, host casts to f32
  - PSUM banks 3/3/1/1 (FL/S/V/out): deep FL + S pipelining is worth
    more than double-buffered V/out
  - tail chunk (48 nodes) is software-pipelined into chunk 12
"""

import sys

sys.path.insert(0, "/opt/trn_rl_repo")

import numpy as np
import ml_dtypes

import concourse.bass as bass
import concourse.bacc as bacc
import concourse.mybir as mybir
import concourse.tile as tile
from concourse.bass_utils import run_bass_kernel_spmd

BF16 = ml_dtypes.bfloat16

B, K, N, F, H = 20000, 10, 100000, 256, 128
NCORES = 8
B_CORE = B // NCORES                # 2500
CHUNK_NODES = 192                   # nodes per chunk
CHUNK_ROWS = CHUNK_NODES * K        # 1920
NCHUNK = 13                         # full chunks; + 1 tail chunk of 48 nodes
TAIL_NODES = 48                     # 1 quad (4 groups of 12)
TAIL_ROWS = 512                     # one gather (48*K=480 used, 512 padded)
PAD_NODES = NCHUNK * CHUNK_NODES + TAIL_NODES   # 2544
PAD_ROWS = NCHUNK * CHUNK_ROWS + TAIL_ROWS      # 25472
IDX_COLS = NCHUNK * (CHUNK_ROWS // 16) + TAIL_ROWS // 16  # 1592
IDX0_COLS = CHUNK_ROWS // 16        # 120 (chunk 0, separate tensor)
GSIZES = (512, 512, 512, 384)       # per-chunk gather sizes (sum 1920)
GOFFS = (0, 512, 1024, 1536)
GROUP_NODES = 12                    # nodes per attention group
GROUP_ROWS = GROUP_NODES * K        # 120
QUADS = 4                           # quads per chunk (4 groups each)
TBL_ROWS = 25600                    # per-core compact table rows (padded)
MASK_L = 50.0

_CACHED = {}


def _build_program():
    nc = bacc.Bacc(
        "TRN2",
        target_bir_lowering=False,
        debug=False,
        num_devices=NCORES,
        num_swdge_queues=4,
    )
    dt = mybir.dt
    f32, bf16, i16 = dt.float32, dt.bfloat16, dt.int16
    Tanh = mybir.ActivationFunctionType.Tanh
    Exp = mybir.ActivationFunctionType.Exp

    table = nc.dram_tensor("table", [TBL_ROWS, F], bf16, kind="ExternalInput")
    # chunks 0/1 arrive pre-gathered+transposed from the host so the PE can
    # start before the SWDGE library load (~17.5us) and first gather finish
    x0t = nc.dram_tensor("x0t", [128, 2, CHUNK_ROWS], bf16,
                         kind="ExternalInput")
    x1t = nc.dram_tensor("x1t", [128, 2, CHUNK_ROWS], bf16,
                         kind="ExternalInput")
    idxs1 = nc.dram_tensor("idxs1", [128, IDX_COLS - 2 * IDX0_COLS], i16,
                           kind="ExternalInput")
    # packed constants: 8 weight mats (needed first) | um | vm | blockind
    wnames = ["w1a0", "w1a1", "w2a0", "w2a1", "w3a0", "w3a1", "w3b", "cmt"]
    WREST = 128 + 492 + 32
    wpk_w = nc.dram_tensor("wpk_w", [128, 8 * 128], bf16,
                           kind="ExternalInput")
    wpk_r = nc.dram_tensor("wpk_r", [128, WREST], bf16, kind="ExternalInput")
    # bf16 output staging: halves the DVE copy cost and store bytes; the
    # host casts back to f32 (adds ~0.17% rel err, within tolerance)
    out = nc.dram_tensor("out", [PAD_NODES, H], bf16, kind="ExternalOutput")

    with tile.TileContext(nc) as tc:
        with (
            tc.tile_pool(name="consts", bufs=1) as cpool,
            tc.tile_pool(name="xt", bufs=3) as xpool,
            tc.tile_pool(name="tp", bufs=3) as tpool,
            tc.tile_pool(name="attn", bufs=4) as apool,
            tc.tile_pool(name="psbig", bufs=3, space="PSUM") as psbig,
            tc.tile_pool(name="pss", bufs=3, space="PSUM") as pssp,
            tc.tile_pool(name="psv", bufs=1, space="PSUM") as psvp,
            tc.tile_pool(name="pso", bufs=1, space="PSUM") as psop,
        ):
            # ---- load constants to SBUF ----
            # startup-critical tensors ride both HWDGE rings, piece by piece,
            # ordered by first use: chunk-0 x pieces + weights on ACT ring,
            # mask consts + chunk-1 x pieces + gather indices on SP ring
            x0sb = cpool.tile([128, 2, CHUNK_ROWS], bf16, tag="c_x0")
            x1sb = cpool.tile([128, 2, CHUNK_ROWS], bf16, tag="c_x1")
            wp = cpool.tile([128, 8 * 128 + WREST], bf16, tag="c_wpack")
            nc.scalar.dma_start(out=x0sb[:, :, 0:512], in_=x0t[:, :, 0:512])
            nc.scalar.dma_start(out=wp[:, 0:1024], in_=wpk_w[:, :])
            nc.sync.dma_start(out=wp[:, 1024:], in_=wpk_r[:, :])
            for o, w in zip(GOFFS[1:], GSIZES[1:]):
                nc.scalar.dma_start(out=x0sb[:, :, o:o + w],
                                    in_=x0t[:, :, o:o + w])
            for o, w in zip(GOFFS, GSIZES):
                nc.sync.dma_start(out=x1sb[:, :, o:o + w],
                                  in_=x1t[:, :, o:o + w])
            idx1_sb = cpool.tile([128, IDX_COLS - 2 * IDX0_COLS], i16,
                                 tag="c_idx1")
            nc.sync.dma_start(out=idx1_sb[:, :], in_=idxs1[:, :])
            wsb = {n: wp[:, 128 * i:128 * (i + 1)]
                   for i, n in enumerate(wnames)}
            um_sb = wp[0:13, 1024:1152]
            vm_sb = wp[0:13, 1152:1644]
            bi_sb = wp[:, 1644:1676]

            # hoisted num_idxs registers (one MOVE each instead of per-gather)
            nregs = {sz: nc.gpsimd.to_reg(sz) for sz in (512, 384)}

            def gather(xt, idx_sb, icol0, gsz, queue):
                """Gather gsz rows into the full tile xt [128, 2, gsz]."""
                nc.gpsimd.dma_gather(
                    out_ap=xt[:, :, :],
                    in_ap=table[:, :],
                    idxs_ap=idx_sb[:, icol0:icol0 + gsz // 16],
                    num_idxs=gsz,
                    num_idxs_reg=nregs[gsz],
                    elem_size=F,
                    transpose=True,
                    queue_num=queue,
                )

            def first_layer(xts, ts, widths=GSIZES, offs=GOFFS):
                """T_j = tanh(W_ja^T X^T) and G = CmT T1.

                Pieces-outer order: each gather piece is fully consumed
                (t1/t2/t3/G) before the next, so compute starts as soon as
                the first piece lands and G is ready early for the quads.
                """
                t1, t2, t3, g = ts
                for xt, w, to in zip(xts, widths, offs):
                    for w0, w1, tj in (("w1a0", "w1a1", t1),
                                       ("w2a0", "w2a1", t2),
                                       ("w3a0", "w3a1", t3)):
                        ps = psbig.tile([128, 512], f32, tag="psb")
                        nc.tensor.matmul(ps[:, 0:w], wsb[w0][:, :],
                                         xt[:, 0, 0:w],
                                         start=True, stop=False)
                        nc.tensor.matmul(ps[:, 0:w], wsb[w1][:, :],
                                         xt[:, 1, 0:w],
                                         start=False, stop=True)
                        nc.scalar.activation(tj[:, to:to + w], ps[:, 0:w],
                                             Tanh)
                    psg = psbig.tile([128, 512], f32, tag="psb")
                    nc.tensor.matmul(psg[:, 0:w], wsb["cmt"][:, :],
                                     t1[:, to:to + w],
                                     start=True, stop=True)
                    nc.vector.tensor_copy(g[:, to:to + w], psg[:, 0:w])

            def quad(t2t, t3t, gt, outst, q, row_base):
                """One quad: 4 groups of 12 nodes starting at row_base."""
                ps_s = pssp.tile([128, 492], f32, tag="pss")
                # mask bias first (start=True over all 480 cols)
                nc.tensor.matmul(ps_s[:, 0:492], um_sb[:, :], vm_sb[:, :],
                                 start=True, stop=False,
                                 skip_group_check=True)
                for g4 in range(4):
                    r0 = row_base + GROUP_ROWS * g4
                    nc.tensor.matmul(
                        ps_s[:, 120 * g4:120 * g4 + 120],
                        gt[:, r0:r0 + 128],
                        t2t[:, r0:r0 + 120],
                        start=False, stop=True, skip_group_check=True)
                em = apool.tile([128, 488], bf16, tag="em")
                # cols 480:488 hold exp(0)=1 from the mask matmul's zero
                # columns; they only feed unused ct partitions 120:127
                nc.scalar.activation(em[0:120, 0:488], ps_s[0:120, 0:488],
                                     Exp)
                rq = apool.tile([128, 4], f32, tag="rq")
                nc.vector.reduce_sum(
                    rq[0:120, :],
                    em[0:120, 0:480].rearrange("p (g j) -> p g j", g=4),
                    axis=mybir.AxisListType.X)
                rec = apool.tile([128, 4], bf16, tag="rec")
                with nc.allow_low_precision(
                        reason="1/r feeds a bf16 matmul; bf16 rounding of the "
                               "per-query softmax scale is within tolerance"):
                    nc.vector.reciprocal(rec[0:120, :], rq[0:120, :])

                # V blocks (PE transpose + W3b) and column weights c
                ps_v = psvp.tile([128, 4, 128], f32, tag="psv")
                ct_ps = ps_s[:, 488:492]
                for g4 in range(4):
                    r0 = row_base + GROUP_ROWS * g4
                    nc.tensor.matmul(ps_v[:, g4, :], t3t[:, r0:r0 + 128],
                                     wsb["w3b"][:, :],
                                     start=True, stop=True)
                    nc.tensor.matmul(ct_ps[:, g4:g4 + 1],
                                     em[0:120, 120 * g4:120 * g4 + 128],
                                     rec[0:120, g4:g4 + 1],
                                     start=True, stop=True,
                                     skip_group_check=True)
                ctsb = apool.tile([128, 4], f32, tag="ctsb")
                nc.vector.tensor_copy(ctsb[0:120, :], ct_ps[0:120, :])
                # M = V * c in one DVE op: c broadcast along h via a
                # 0-stride inner dim (walrus rejects a PSUM-sourced in1,
                # so c stages through SBUF first)
                m4 = apool.tile([128, 4, 128], bf16, tag=f"m4_{q}",
                                name="m4")
                with nc.allow_low_precision(
                        reason="M feeds a bf16 matmul; same rounding class "
                               "as a tensor_scalar scale-copy"):
                    nc.vector.tensor_tensor(
                        m4[0:120, :, :], ps_v[0:120, :, :],
                        ctsb[0:120, :].unsqueeze(2).broadcast_to(
                            (120, 4, 128)),
                        mybir.AluOpType.mult)
                # whole-tile read after the scale: empirically decouples the
                # next quad's V-matmuls from the m4 consumer chain
                vg = apool.tile([128, 4], f32, tag="vg")
                nc.vector.tensor_copy(vg[0:120, :], ps_v[0:120, :, 0])
                return m4

            def out_stage(m4, outst, q):
                # one 512-col out-matmul: bi maps row j -> node j//10 the
                # same way in every group, so all 4 groups share one lhsT.
                # Deferred to chunk end so the copy doesn't head-of-line
                # block the next quad's chain ops on the DVE queue.
                ps_o = psop.tile([32, 4, 128], f32, tag="pso")
                nc.tensor.matmul(ps_o[:, :, :],
                                 bi_sb[0:120, :],
                                 m4[0:120, :, :],
                                 start=True, stop=True,
                                 skip_group_check=True)
                with nc.allow_low_precision(
                        reason="bf16 output staging; host casts to f32"):
                    nc.vector.tensor_copy(
                        outst[0:32, q, :, :], ps_o[:, :, :])

            # zero the +8 pad cols read by the last group's 128-wide lhsT;
            # done once per pool buffer (tanh never writes past CHUNK_ROWS)
            for _ in range(3):
                t3z = tpool.tile([128, CHUNK_ROWS + 8], bf16, tag="t3")
                gz = tpool.tile([128, CHUNK_ROWS + 8], bf16, tag="g")
                nc.vector.memset(t3z[:, CHUNK_ROWS:], 0.0)
                nc.vector.memset(gz[:, CHUNK_ROWS:], 0.0)

            ttiles = None
            for c in range(NCHUNK):
                # ---- chunk input: preloaded (c<2) or 4 SWDGE gathers ----
                if c < 2:
                    xsb = x0sb if c == 0 else x1sb
                    xts = [xsb[:, :, to:to + w]
                           for w, to in zip(GSIZES, GOFFS)]
                else:
                    xts = [xpool.tile([128, 2, w], bf16, tag=f"xt{ti}",
                                      name=f"xt{ti}")
                           for ti, w in enumerate(GSIZES)]
                    icol = (c - 2) * 120
                    for ti, (xt, w, to) in enumerate(zip(xts, GSIZES,
                                                         GOFFS)):
                        gather(xt, idx1_sb, icol + to // 16, w, ti)
                if c == NCHUNK - 1:
                    # tail-chunk gather rides along with the last full chunk
                    xtt = xpool.tile([128, 2, 512], bf16, tag="xtt")
                    gather(xtt, idx1_sb, (NCHUNK - 2) * 120, 512, 0)

                # ---- first layers: T_j^T = tanh(W_ja^T X^T), G ----
                t1 = tpool.tile([128, CHUNK_ROWS + 8], bf16, tag="t1")
                t2 = tpool.tile([128, CHUNK_ROWS + 8], bf16, tag="t2")
                t3 = tpool.tile([128, CHUNK_ROWS + 8], bf16, tag="t3")
                g = tpool.tile([128, CHUNK_ROWS + 8], bf16, tag="g")
                first_layer(xts, (t1, t2, t3, g))
                if c == NCHUNK - 1:
                    # tail-chunk first layer, emitted before the last quads
                    # so the PE has fill work during their serial chain
                    tt1 = tpool.tile([128, 520], bf16, tag="tt1")
                    tt2 = tpool.tile([128, 520], bf16, tag="tt2")
                    tt3 = tpool.tile([128, 520], bf16, tag="tt3")
                    tg = tpool.tile([128, 520], bf16, tag="tg")
                    first_layer([xtt], (tt1, tt2, tt3, tg),
                                widths=(512,), offs=(0,))
                    ttiles = (tt2, tt3, tg)

                # ---- attention: 4 quads of 4 groups ----
                # node layout: slab g4 holds partitions 32g4..32g4+12, one
                # 128-col block per quad; host permutes nodes so slab g4 maps
                # to contiguous out rows [48g4, 48(g4+1)) of the chunk
                outst = apool.tile([32, QUADS, 4, H], bf16, tag="outst")
                m4s = [quad(t2, t3, g, outst, q, 480 * q)
                       for q in range(QUADS)]
                for q in range(QUADS):
                    out_stage(m4s[q], outst, q)

                # ---- store chunk output: one DMA per 12-row slab ----
                for g4 in range(4):
                    dst = out[c * CHUNK_NODES + 48 * g4:
                              c * CHUNK_NODES + 48 * (g4 + 1), :].rearrange(
                        "(q i) d -> i q d", i=GROUP_NODES)
                    src = outst[0:GROUP_NODES, :, g4, :]
                    nc.sync.dma_start(out=dst, in_=src)

            # ---- tail chunk: 48 real+pad nodes (1 quad) ----
            tt2, tt3, tg = ttiles
            toutst = apool.tile([32, QUADS, 4, H], bf16, tag="outst")
            tm4 = quad(tt2, tt3, tg, toutst, 0, 0)
            out_stage(tm4, toutst, 0)
            for g4 in range(4):
                dst = out[NCHUNK * CHUNK_NODES + GROUP_NODES * g4:
                          NCHUNK * CHUNK_NODES + GROUP_NODES * (g4 + 1), :]
                nc.scalar.dma_start(
                    out=dst,
                    in_=toutst[0:GROUP_NODES, 0, g4, :])

    nc.finalize()
    return nc


def _host_prep(neighbors, embed_table, W1a, W1b, W2a, W2b, W3a, W3b):
    """Shard + build per-core input maps."""
    embed_table = np.asarray(embed_table)
    ebf = np.ascontiguousarray(embed_table.astype(BF16))

    def b(x):
        return np.ascontiguousarray(np.asarray(x).astype(BF16))

    w1a, w2a, w3a = (np.asarray(w, np.float32) for w in (W1a, W2a, W3a))
    wmats = [
        w1a[0:128], w1a[128:256], w2a[0:128], w2a[128:256],
        w3a[0:128], w3a[128:256], np.asarray(W3b, np.float32),
        np.asarray(W1b, np.float32) @ np.asarray(W2b, np.float32).T,
    ]
    # mask = U @ Vm^T adds 0 in-block, -L off-block (rank 13)
    bi = np.zeros((120, GROUP_NODES), np.float32)
    for p in range(120):
        bi[p, p // K] = 1.0
    um = np.zeros((128, 128), np.float32)
    um[0:12, 0:120] = bi.T
    um[12, 0:120] = 1.0
    vm = np.zeros((128, 492), np.float32)
    for qq in range(4):
        vm[0:12, 120 * qq:120 * (qq + 1)] = MASK_L * bi.T
        vm[12, 120 * qq:120 * (qq + 1)] = -MASK_L
    bi128 = np.zeros((128, 32), np.float32)
    bi128[0:120, 0:GROUP_NODES] = bi
    wpk_w_arr = np.concatenate(wmats, axis=1)
    wpk_r_arr = np.concatenate([um, vm, bi128], axis=1)
    shared = {"wpk_w": b(wpk_w_arr), "wpk_r": b(wpk_r_arr)}

    nbr = np.asarray(neighbors).astype(np.int64)
    in_maps = []
    for c in range(NCORES):
        nb_c = nbr[c * B_CORE:(c + 1) * B_CORE]           # [2500, 10]
        uniq, inv = np.unique(nb_c, return_inverse=True)
        assert uniq.size <= TBL_ROWS
        tbl = np.zeros((TBL_ROWS, F), BF16)
        tbl[:uniq.size] = ebf[uniq]
        # permute nodes within full chunks so the packed-psum output slabs
        # land on contiguous out rows: slot 12*(4q+qq)+j <- node 48qq+12q+j
        perm = np.empty(CHUNK_NODES, np.int64)
        for pq in range(4):
            for pqq in range(4):
                for pj in range(GROUP_NODES):
                    perm[GROUP_NODES * (4 * pq + pqq) + pj] = \
                        48 * pqq + GROUP_NODES * pq + pj
        nodes = np.zeros((PAD_NODES, K), np.int16)
        nodes[:B_CORE] = inv.astype(np.int16).reshape(B_CORE, K)
        for ch in range(NCHUNK):
            blk = nodes[ch * CHUNK_NODES:(ch + 1) * CHUNK_NODES].copy()
            nodes[ch * CHUNK_NODES:(ch + 1) * CHUNK_NODES] = blk[perm]
        flat = np.zeros(PAD_ROWS, np.int16)
        flat[:nodes.size] = nodes.ravel()
        # wrap: index j of a chunk at [j % 16, j // 16], replicated to 128
        idx128 = np.zeros((128, IDX_COLS), np.int16)
        col = row = 0
        for sz in [CHUNK_ROWS] * NCHUNK + [TAIL_ROWS]:
            blk = flat[row:row + sz].reshape(sz // 16, 16).T
            idx128[:, col:col + sz // 16] = np.tile(blk, (8, 1))
            row += sz
            col += sz // 16
        # pre-gather + transpose chunks 0/1 on the host: [128, 2, 1920]
        xts_host = []
        for ch in range(2):
            rows = nodes[ch * CHUNK_NODES:(ch + 1) * CHUNK_NODES].ravel()
            xg = tbl[rows.astype(np.int64)]            # [1920, 256] bf16
            xts_host.append(np.ascontiguousarray(
                xg.reshape(CHUNK_ROWS, 2, 128).transpose(2, 1, 0)))
        in_maps.append({
            "table": tbl,
            "x0t": xts_host[0],
            "x1t": xts_host[1],
            "idxs1": np.ascontiguousarray(idx128[:, 2 * IDX0_COLS:]),
            **{k: v for k, v in shared.items()},
        })
    return in_maps


def kernel(neighbors, embed_table, W1a, W1b, W2a, W2b, W3a, W3b, _trace=False,
           **trace_kwargs):
    key = "prog"
    if key not in _CACHED:
        _CACHED[key] = _build_program()
    nc = _CACHED[key]
    in_maps = _host_prep(neighbors, embed_table, W1a, W1b, W2a, W2b, W3a, W3b)
    res = None
    for attempt in range(3):
        try:
            res = run_bass_kernel_spmd(nc, in_maps, list(range(NCORES)),
                                       trace=_trace, **trace_kwargs)
            break
        except Exception:
            # the axon/TRN2 device occasionally wedges transiently
            # (NRT_EXEC_UNIT_UNRECOVERABLE / INTERNAL); a retry recovers
            if attempt == 2:
                raise
            import time
            time.sleep(5)
    outs = [res.results[c]["out"][:B_CORE] for c in range(NCORES)]
    full = np.concatenate(outs, axis=0).astype(np.float32)
    kernel.last_results = res
    return full
